# revision 1
# baseline (speedup 1.0000x reference)
"""GCN EndNodeSelector Bass kernel for TRN2, 8-core SPMD.

Pipeline (per core, nodes row-sharded, degree-sorted within core):
  P1: z1 = x @ W1 (PE, xT streamed from DRAM in fp8-e4m3 with W1 pre-scaled
      x64, compensated in the dinv scale), z1' = dinv * z1
  P2: AllGather z1' -> z1g in LOCAL dram (per-core chunks of NPC+1 rows;
      last row zero)
  P3: conv1 aggregation via dma_gather (LO/HI windows, byte-balanced over
      the 4 SWDGE queues) + strided tensor_reduce; h1 = dinv*sum + b1;
      u = elu(h1). Tail: extract this core's mapping-winner u-rows into a
      compact buffer wb [WMAX, H] via a small local gather.
  P4: AllGather wb -> wg (compact winner table, ~1.3MB instead of 12.8MB)
  P5: x1 gather from wg (one call, int16-safe), hcat=[u|x1u],
      z2 = hcat @ W2
  P6: AllGather z2' -> z2g (Local)
  P7: conv2 aggregation, h2, e=elu(h2), v = e.fc_w + fc_b, mask,
      S = allreduce(sum(exp(v-48))), y = v - 48 - ln(S)

dma_gather needs int16 indices, so the gathered table is addressed through
two windows split at a core boundary (each < 32768 rows). Every per-core
chunk carries one guaranteed-zero row used for slot padding. Do NOT use
single_packet=True (crashes the exec unit).
"""
import sys
import numpy as np

sys.path.insert(0, '/opt/trn_rl_repo')

import concourse.bass as bass
import concourse.bacc as bacc
import concourse.tile as tile
from concourse import mybir
from concourse import bass_utils
from concourse.masks import make_identity

F32 = mybir.dt.float32
I16 = mybir.dt.int16
AX = mybir.AxisListType
ALU = mybir.AluOpType
ACTF = mybir.ActivationFunctionType

P = 128
H = 64
BIG_NEG = -1e9
SOFTMAX_SHIFT = 48.0
GMAX_K = 48          # max slots per gather call (per-partition buffer budget)


def _wrap_idx(flat):
    """dma_gather index layout: [128, ceil(n/16)] int16, list wrapped into 16
    partitions (i -> [i%16, i//16]) and replicated across the 8 Q7 groups."""
    flat = np.asarray(flat, dtype=np.int64)
    n = flat.size
    s = (n + 15) // 16
    pad = np.full(s * 16, -1, dtype=np.int64)
    pad[:n] = flat
    assert pad.max() < 32768
    arr = pad.reshape(s, 16).T.astype(np.int16)     # [16, s]
    return np.tile(arr, (8, 1))                      # [128, s]


# ---------------------------------------------------------------------------
# Host preprocessing
# ---------------------------------------------------------------------------

def host_prep(x, edge_index, all_edge_index, s_mapping_index, e_mask, C):
    N, F = x.shape
    n_per = N // C
    NPC = ((n_per + P - 1) // P) * P
    NT = NPC // P
    NPC1 = NPC + 1                       # +1 zero row per core chunk
    ZLOC = NPC                           # zero row local index within chunk
    FP = ((F + P - 1) // P) * P
    C_LO = min(C - 1, 32767 // NPC1)
    assert C_LO >= 1 and (C - C_LO) * NPC1 <= 32768

    src = np.asarray(edge_index[0], dtype=np.int64)
    dst = np.asarray(edge_index[1], dtype=np.int64)
    deg = np.bincount(dst, minlength=N).astype(np.int64) + 1

    # Self-loops are NOT gathered: z1'[v] is kept core-local in SBUF and
    # added via DVE before the dinv scale. Tables hold real neighbors only.
    core_of = np.arange(N) // n_per
    is_lo_src = core_of[src] < C_LO
    nlo = np.bincount(dst[is_lo_src], minlength=N)
    nhi = (deg - 1) - nlo

    def tile_cost(perm_all):
        tot = 0
        for arrs in (nlo, nhi):
            kt = np.zeros(NT, dtype=np.int64)
            for c in range(C):
                a = np.zeros(NPC, dtype=np.int64)
                a[:n_per] = arrs[perm_all[c]]
                kt = np.maximum(kt, a.reshape(NT, P).max(axis=1))
            tot += kt.sum()
        return tot

    cands = []
    for key in ('deg', 'lohi'):
        lp = np.empty((C, n_per), dtype=np.int64)
        for c in range(C):
            ids = np.arange(c * n_per, (c + 1) * n_per)
            if key == 'deg':
                o = np.argsort(-deg[ids], kind='stable')
            else:
                o = np.lexsort((-nhi[ids], -nlo[ids]))
            lp[c] = ids[o]
        cands.append((tile_cost(lp), lp))
    cands.sort(key=lambda t: t[0])
    lperm = cands[0][1]

    inv_row = np.empty(N, dtype=np.int64)
    for c in range(C):
        inv_row[lperm[c]] = c * NPC1 + np.arange(n_per)

    def per_tile_max(arrs):
        kt = np.zeros(NT, dtype=np.int64)
        for c in range(C):
            a = np.zeros(NPC, dtype=np.int64)
            a[:n_per] = arrs[lperm[c]]
            kt = np.maximum(kt, a.reshape(NT, P).max(axis=1))
        return kt

    K_lo = np.maximum(per_tile_max(nlo), 1)
    K_hi = np.maximum(per_tile_max(nhi), 1)

    es = np.argsort(dst, kind='stable')
    dst_sorted = dst[es]
    src_sorted = src[es]
    starts = np.searchsorted(dst_sorted, np.arange(N))
    ends = np.searchsorted(dst_sorted, np.arange(N) + 1)

    ZROW_LO = ZLOC                        # chunk 0's zero row (lo window)
    ZROW_HI = ZLOC                        # chunk C_LO's zero row (hi local)
    HI_BASE = C_LO * NPC1

    slots_lo = [[np.full((P, int(K_lo[t])), ZROW_LO, np.int64)
                 for t in range(NT)] for _ in range(C)]
    slots_hi = [[np.full((P, int(K_hi[t])), ZROW_HI, np.int64)
                 for t in range(NT)] for _ in range(C)]
    for c in range(C):
        for t in range(NT):
            base = t * P
            nreal = min(P, max(0, n_per - base))
            for pp in range(nreal):
                v = lperm[c, base + pp]
                srcs = src_sorted[starts[v]:ends[v]]
                rows = inv_row[srcs]
                lo = rows[rows < HI_BASE]
                hi = rows[rows >= HI_BASE] - HI_BASE
                slots_lo[c][t][pp, :lo.size] = lo
                slots_hi[c][t][pp, :hi.size] = hi

    groups = []          # (t0, t1, sum_klo, sum_khi)
    t0 = 0
    while t0 < NT:
        t1 = t0
        sk = 0
        while t1 < NT and sk + max(int(K_lo[t1]), int(K_hi[t1])) <= GMAX_K:
            sk += max(int(K_lo[t1]), int(K_hi[t1]))
            t1 += 1
        if t1 == t0:
            t1 = t0 + 1
        groups.append((t0, t1,
                       int(K_lo[t0:t1].sum()), int(K_hi[t0:t1].sum())))
        t0 = t1

    def build_wrapped(slotsets):
        per_core = []
        col_offs = None
        for c in range(C):
            parts = []
            for (a, b, _, _) in groups:
                flat = np.concatenate(
                    [slotsets[c][t].T.reshape(-1) for t in range(a, b)])
                # flat order within group: tile-major, slot k, partition p
                parts.append(_wrap_idx(flat))
            col_offs = np.cumsum([0] + [q.shape[1] for q in parts])
            per_core.append(np.concatenate(parts, axis=1))
        return np.stack(per_core), col_offs

    itbl_lo, lo_coffs = build_wrapped(slots_lo)
    itbl_hi, hi_coffs = build_wrapped(slots_hi)

    # x1 winner emulation (last-write-wins)
    map_vec = np.full(N, -1, dtype=np.int64)
    map_vec[np.asarray(s_mapping_index[1], dtype=np.int64)] = np.asarray(
        s_mapping_index[0], dtype=np.int64)
    gen = map_vec[np.asarray(all_edge_index[0], dtype=np.int64)]
    valid = gen >= 0
    tgt = np.asarray(all_edge_index[1], dtype=np.int64)[valid]
    genv = gen[valid]
    x1_src = np.full(N, -1, dtype=np.int64)
    if tgt.size:
        u_t, first_rev = np.unique(tgt[::-1], return_index=True)
        x1_src[u_t] = genv[tgt.size - 1 - first_rev]

    # Compact winner table: per-core sorted unique local rows of winner nodes.
    # wg = concat_c(pad(wlist_c, WMAX, ZLOC)); x1 gathers wg[cidx] with
    # cidx = c_g*WMAX + pos. Dead dsts -> core0's last pad slot (zeros).
    winners = np.unique(x1_src[x1_src >= 0])
    wcore = winners // n_per  # owner core in ORIGINAL numbering? no: via lperm
    # owner core/local-row come from inv_row (lperm space)
    wrow = inv_row[winners]               # c*NPC1 + local_row
    wcore = wrow // NPC1
    wloc = wrow % NPC1
    wlists = [np.sort(wloc[wcore == c]) for c in range(C)]
    maxlen = max((len(w) for w in wlists), default=0)
    WMAX = ((maxlen + 1 + 127) // 128) * 128
    pos_of = {}
    wext = np.empty((C, P, WMAX // 16), dtype=np.int16)
    for c in range(C):
        lst = wlists[c]
        for i, l in enumerate(lst):
            pos_of[c * NPC1 + int(l)] = c * WMAX + i
        flat = np.full(WMAX, ZLOC, np.int64)
        flat[:len(lst)] = lst
        wext[c] = _wrap_idx(flat)[:, :WMAX // 16]

    DEAD = 0 * WMAX + (WMAX - 1)          # core0 pad slot => zeros
    x1wc = np.empty((C, P, 8 * NT), dtype=np.int16)
    for c in range(C):
        flat = np.full(NT * P, DEAD, np.int64)
        for t in range(NT):
            base = t * P
            nreal = min(P, max(0, n_per - base))
            for pp in range(nreal):
                g = x1_src[lperm[c, base + pp]]
                if g >= 0:
                    flat[t * P + pp] = pos_of[int(inv_row[g])]
        assert flat.max() < 32768
        x1wc[c] = _wrap_idx(flat)

    degt = np.ones((C, P, NT), dtype=np.float32)
    keep = np.zeros((C, P, NT), dtype=np.float32)
    mneg = np.full((C, P, NT), np.float32(BIG_NEG), dtype=np.float32)
    maskf = np.asarray(e_mask).reshape(-1).astype(bool)
    for c in range(C):
        dp = np.ones(NPC, dtype=np.float32)
        dp[:n_per] = deg[lperm[c]].astype(np.float32)
        kp = np.zeros(NPC, dtype=np.float32)
        kp[:n_per] = (~maskf[lperm[c]]).astype(np.float32)
        mp = np.full(NPC, np.float32(BIG_NEG), dtype=np.float32)
        mp[:n_per] = np.where(maskf[lperm[c]], np.float32(BIG_NEG),
                              np.float32(0.0))
        degt[c] = dp.reshape(NT, P).T
        keep[c] = kp.reshape(NT, P).T
        mneg[c] = mp.reshape(NT, P).T

    xts = []
    xf = np.asarray(x, dtype=np.float32)
    for c in range(C):
        xt = np.zeros((FP, NPC), dtype=np.float32)
        xt[:F, :n_per] = xf[lperm[c]].T
        xts.append(xt)

    tot_slots = int((K_lo.sum() + K_hi.sum()) * P)
    real_slots = int(E + N) // C if (E := len(src)) else 0
    meta = dict(N=N, F=F, C=C, n_per=n_per, NPC=NPC, NPC1=NPC1, NT=NT,
                tot_slots=tot_slots, real_slots=real_slots,
                FP=FP, C_LO=C_LO, HI_BASE=HI_BASE, WMAX=WMAX,
                K_lo=K_lo, K_hi=K_hi, groups=groups,
                lo_coffs=list(lo_coffs), hi_coffs=list(hi_coffs),
                SUMW_LO=itbl_lo.shape[2], SUMW_HI=itbl_hi.shape[2],
                lperm=lperm)
    return dict(xts=xts, itbl_lo=itbl_lo, itbl_hi=itbl_hi,
                wext=wext, x1wc=x1wc,
                degt=degt, keep=keep, mneg=mneg), meta


def host_prep_weights(conv1_w, conv1_b, conv2_w, conv2_b, fc_w, fc_b, meta):
    FP = meta['FP']
    F = meta['F']
    w1 = np.zeros((FP, H), dtype=np.float32)
    w1[:F] = np.asarray(conv1_w, dtype=np.float32)
    w2 = np.asarray(conv2_w, dtype=np.float32)
    b1r = np.broadcast_to(np.asarray(conv1_b, np.float32), (P, H)).copy()
    b2r = np.broadcast_to(np.asarray(conv2_b, np.float32), (P, H)).copy()
    fcwr = np.broadcast_to(np.asarray(fc_w, np.float32).reshape(1, H),
                           (P, H)).copy()
    fcbr = np.full((P, 1), np.float32(np.asarray(fc_b).reshape(-1)[0]),
                   np.float32)
    return w1, w2, b1r, b2r, fcwr, fcbr


# ---------------------------------------------------------------------------
# Device kernel
# ---------------------------------------------------------------------------

def build_kernel(meta, mm_dtype=F32, debug_taps=False, skip_b1=False,
                 skip_b2=False):
    C = meta['C']
    NPC = meta['NPC']
    NPC1 = meta['NPC1']
    NT = meta['NT']
    FP = meta['FP']
    C_LO = meta['C_LO']
    K_lo = meta['K_lo']
    K_hi = meta['K_hi']
    groups = meta['groups']
    lo_coffs = meta['lo_coffs']
    hi_coffs = meta['hi_coffs']
    SUMW_LO = meta['SUMW_LO']
    SUMW_HI = meta['SUMW_HI']
    NF = FP // P
    MBLK = 512
    n_mblk = (NPC + MBLK - 1) // MBLK
    NLO = C_LO * NPC1
    NHI = (C - C_LO) * NPC1
    GK_LO = max(g[2] for g in groups)
    GK_HI = max(g[3] for g in groups)

    nc = bacc.Bacc("TRN2", target_bir_lowering=False, debug=False,
                   num_devices=C, num_swdge_queues=4)

    xt_d = nc.dram_tensor("xt", [FP, NPC], mm_dtype, kind="ExternalInput")
    w1_d = nc.dram_tensor("w1", [FP, H], mm_dtype, kind="ExternalInput")
    w2_d = nc.dram_tensor("w2", [2 * H, H], F32, kind="ExternalInput")
    b1_d = nc.dram_tensor("b1r", [P, H], F32, kind="ExternalInput")
    b2_d = nc.dram_tensor("b2r", [P, H], F32, kind="ExternalInput")
    fcw_d = nc.dram_tensor("fcwr", [P, H], F32, kind="ExternalInput")
    fcb_d = nc.dram_tensor("fcbr", [P, 1], F32, kind="ExternalInput")
    ilo_d = nc.dram_tensor("itbl_lo", [P, SUMW_LO], I16, kind="ExternalInput")
    ihi_d = nc.dram_tensor("itbl_hi", [P, SUMW_HI], I16, kind="ExternalInput")
    WMAX = meta['WMAX']
    wext_d = nc.dram_tensor("wext", [P, WMAX // 16], I16, kind="ExternalInput")
    x1wc_d = nc.dram_tensor("x1wc", [P, 8 * NT], I16, kind="ExternalInput")
    degt_d = nc.dram_tensor("degt", [P, NT], F32, kind="ExternalInput")
    keep_d = nc.dram_tensor("keep", [P, NT], F32, kind="ExternalInput")
    mneg_d = nc.dram_tensor("mneg", [P, NT], F32, kind="ExternalInput")
    y_d = nc.dram_tensor("y", [P, NT], F32, kind="ExternalOutput")
    if debug_taps:
        z1dbg_d = nc.dram_tensor("z1dbg", [NPC1, H], F32, kind="ExternalOutput")
        udbg_d = nc.dram_tensor("udbg", [NPC1, H], F32, kind="ExternalOutput")
        z2dbg_d = nc.dram_tensor("z2dbg", [NPC1, H], F32, kind="ExternalOutput")
        vdbg_d = nc.dram_tensor("vdbg", [P, NT], F32, kind="ExternalOutput")
        gdbg_d = nc.dram_tensor("gdbg", [P, groups[0][2] * H], F32,
                                kind="ExternalOutput")

    rg = [list(range(C))]

    with tile.TileContext(nc) as tc:
        with tc.tile_pool(name="const", bufs=1) as cpool, \
             tc.tile_pool(name="xin", bufs=4) as xpool, \
             tc.tile_pool(name="work", bufs=2) as wpool, \
             tc.tile_pool(name="gath", bufs=3) as gpool, \
             tc.tile_pool(name="big", bufs=1) as bpool, \
             tc.tile_pool(name="ps", bufs=4, space="PSUM") as pspool, \
             tc.tile_pool(name="psz", bufs=2, space="PSUM") as pszpool, \
             tc.tile_pool(name="dram", bufs=1, space="DRAM") as dpool:

            z1b = dpool.tile([NPC1, H], F32)
            ub = dpool.tile([NPC1, H], F32)
            z2b = dpool.tile([NPC1, H], F32)
            ash = "Shared" if C > 4 else "Local"
            # z1/z2: AllGather direct to Local; u: AllGather to Shared then
            # bulk-copy to Local (A/B comparison of gather-table residency).
            z1g = nc.dram_tensor("z1g_loc", [C * NPC1, H], F32,
                                 kind="Internal", addr_space="Local").ap()
            wb = dpool.tile([WMAX, H], F32)
            wg = nc.dram_tensor("wg_loc", [C * WMAX, H], F32,
                                kind="Internal", addr_space="Local").ap()
            z2g = nc.dram_tensor("z2g_loc", [C * NPC1, H], F32,
                                 kind="Internal", addr_space="Local").ap()
            sj_in = dpool.tile([1, 1], F32)
            sj_out = nc.dram_tensor("sj_sh", [1, 1], F32,
                                    kind="Internal", addr_space=ash).ap()

            ident = cpool.tile([P, P], F32)
            make_identity(nc, ident[:])
            w1_sb = cpool.tile([P, NF * H], mm_dtype)
            nc.sync.dma_start(
                w1_sb[:], w1_d.ap().rearrange("(a p) h -> p a h", p=P))
            w2_sb = cpool.tile([P, H], F32)
            nc.sync.dma_start(w2_sb[:], w2_d.ap())
            b1_sb = cpool.tile([P, H], F32)
            nc.sync.dma_start(b1_sb[:], b1_d.ap())
            b2_sb = cpool.tile([P, H], F32)
            nc.sync.dma_start(b2_sb[:], b2_d.ap())
            fcw_sb = cpool.tile([P, H], F32)
            nc.sync.dma_start(fcw_sb[:], fcw_d.ap())
            fcb_sb = cpool.tile([P, 1], F32)
            nc.sync.dma_start(fcb_sb[:], fcb_d.ap())
            ilo_sb = cpool.tile([P, SUMW_LO], I16)
            nc.sync.dma_start(ilo_sb[:], ilo_d.ap())
            ihi_sb = cpool.tile([P, SUMW_HI], I16)
            nc.sync.dma_start(ihi_sb[:], ihi_d.ap())
            wext_sb = cpool.tile([P, WMAX // 16], I16)
            nc.sync.dma_start(wext_sb[:], wext_d.ap())
            x1wc_sb = cpool.tile([P, 8 * NT], I16)
            nc.sync.dma_start(x1wc_sb[:], x1wc_d.ap())
            degt_sb = cpool.tile([P, NT], F32)
            nc.sync.dma_start(degt_sb[:], degt_d.ap())
            keep_sb = cpool.tile([P, NT], F32)
            nc.sync.dma_start(keep_sb[:], keep_d.ap())
            mneg_sb = cpool.tile([P, NT], F32)
            nc.sync.dma_start(mneg_sb[:], mneg_d.ap())

            dinv_sb = cpool.tile([P, NT], F32)
            nc.vector.reciprocal(dinv_sb[:], degt_sb[:])
            nc.scalar.activation(dinv_sb[:], dinv_sb[:], ACTF.Sqrt)
            # P1 output scale: counteracts the x64 W1 pre-scale in fp8 mode
            mm_scale = 1.0 / 64.0 if mm_dtype == mybir.dt.float8e4 else 1.0
            dinv1_sb = cpool.tile([P, NT], F32)
            nc.vector.tensor_scalar(out=dinv1_sb[:], in0=dinv_sb[:],
                                    scalar1=mm_scale, scalar2=None,
                                    op0=ALU.mult)

            zrow_sb = cpool.tile([1, H], F32)
            nc.vector.memset(zrow_sb[:], 0.0)
            nc.sync.dma_start(z1b[NPC:NPC + 1, :], zrow_sb[:])
            nc.sync.dma_start(ub[NPC:NPC + 1, :], zrow_sb[:])
            nc.sync.dma_start(z2b[NPC:NPC + 1, :], zrow_sb[:])

            ones_sb = cpool.tile([P, 1], F32)
            nc.vector.memset(ones_sb[:], 1.0)
            ones_row = cpool.tile([1, P], F32)
            nc.vector.memset(ones_row[:], 1.0)
            neg48_sb = cpool.tile([P, 1], F32)
            nc.vector.memset(neg48_sb[:], -SOFTMAX_SHIFT)

            u_loc = bpool.tile([P, NT * 2 * H], F32)   # [u | x1u] interleaved
            z1loc = bpool.tile([P, NT * H], F32)       # local z1' (self terms)
            z2loc = bpool.tile([P, NT * H], F32)       # local z2' (self terms)

            def elu_into(dst_ap, src_ap, tmp_pool, fd):
                mn = tmp_pool.tile([P, fd], F32, tag="elu_mn")
                ex = tmp_pool.tile([P, fd], F32, tag="elu_ex")
                nc.vector.tensor_scalar(out=mn[:], in0=src_ap, scalar1=0.0,
                                        scalar2=None, op0=ALU.min)
                nc.scalar.activation(ex[:], mn[:], ACTF.Exp)
                nc.vector.tensor_scalar(out=mn[:], in0=src_ap, scalar1=0.0,
                                        scalar2=None, op0=ALU.max)
                nc.vector.tensor_scalar(out=ex[:], in0=ex[:], scalar1=-1.0,
                                        scalar2=None, op0=ALU.add)
                nc.vector.tensor_tensor(out=dst_ap, in0=mn[:], in1=ex[:],
                                        op=ALU.add)

            qbytes = [0, 0, 0, 0]

            def pick_q(slots):
                q = min(range(4), key=lambda i: qbytes[i])
                qbytes[q] += slots
                return q

            def agg_layer(src_g, out_cb, loc_buf, tap=False):
                for gi, (a, b, sklo, skhi) in enumerate(groups):
                    glo = gpool.tile([P, GK_LO * H], F32, tag="glo")
                    ghi = gpool.tile([P, GK_HI * H], F32, tag="ghi")
                    nlo_idx = sklo * P
                    nhi_idx = skhi * P
                    nc.gpsimd.dma_gather(
                        out_ap=glo[:, :sklo * H].rearrange(
                            "p (c h) -> p c h", h=H),
                        in_ap=src_g[0:NLO, :],
                        idxs_ap=ilo_sb[:, lo_coffs[gi]:lo_coffs[gi + 1]],
                        num_idxs=nlo_idx, num_idxs_reg=nlo_idx, elem_size=H,
                        single_packet=False, queue_num=pick_q(nlo_idx))
                    nc.gpsimd.dma_gather(
                        out_ap=ghi[:, :skhi * H].rearrange(
                            "p (c h) -> p c h", h=H),
                        in_ap=src_g[NLO:NLO + NHI, :],
                        idxs_ap=ihi_sb[:, hi_coffs[gi]:hi_coffs[gi + 1]],
                        num_idxs=nhi_idx, num_idxs_reg=nhi_idx, elem_size=H,
                        single_packet=False, queue_num=pick_q(nhi_idx))
                    if tap and gi == 0:
                        nc.sync.dma_start(gdbg_d.ap()[:], glo[:, :sklo * H])
                    ko = 0
                    kho = 0
                    for t in range(a, b):
                        klo_t = int(K_lo[t])
                        khi_t = int(K_hi[t])
                        ra = wpool.tile([P, H], F32, tag="ra")
                        nc.vector.tensor_reduce(
                            out=ra[:],
                            in_=glo[:, ko * H:(ko + klo_t) * H].rearrange(
                                "p (k h) -> p h k", k=klo_t),
                            op=ALU.add, axis=AX.X)
                        rb = wpool.tile([P, H], F32, tag="rb")
                        nc.vector.tensor_reduce(
                            out=rb[:],
                            in_=ghi[:, kho * H:(kho + khi_t) * H].rearrange(
                                "p (k h) -> p h k", k=khi_t),
                            op=ALU.add, axis=AX.X)
                        nc.vector.tensor_tensor(out=ra[:], in0=ra[:],
                                                in1=rb[:], op=ALU.add)
                        # self-loop term: loc_buf holds dinv*z (already
                        # normalized), added before the dst-side dinv scale
                        nc.vector.tensor_tensor(
                            out=ra[:], in0=ra[:],
                            in1=loc_buf[:, t * H:(t + 1) * H], op=ALU.add)
                        agg = wpool.tile([P, H], F32, tag="agg")
                        nc.vector.tensor_scalar(
                            out=agg[:], in0=ra[:],
                            scalar1=dinv_sb[:, t:t + 1], scalar2=None,
                            op0=ALU.mult)
                        out_cb(t, agg)
                        ko += klo_t
                        kho += khi_t

            # ================= P1: z1 = x @ W1 ===========================
            sc = nc.enter_named_scope("p1", False)[0]
            for mp in range(0, n_mblk, 2):
                nmb = min(2, n_mblk - mp)
                w0 = min(MBLK * nmb, NPC - mp * MBLK)
                zt_list = [pszpool.tile([H, MBLK], F32, tag=f"zt{j}",
                                        name=f"ztp{j}")
                           for j in range(nmb)]
                for f in range(NF):
                    xt_sb = xpool.tile([P, 2 * MBLK], mm_dtype, tag="xt")
                    # alternate HWDGE queues (sync/scalar) for xt streaming
                    eng = nc.sync if f % 2 == 0 else nc.scalar
                    eng.dma_start(
                        xt_sb[:, :w0],
                        xt_d.ap()[f * P:(f + 1) * P,
                                  mp * MBLK:mp * MBLK + w0])
                    for j in range(nmb):
                        mw = min(MBLK, NPC - (mp + j) * MBLK)
                        nc.tensor.matmul(
                            zt_list[j][:, :mw],
                            lhsT=w1_sb[:, f * H:(f + 1) * H],
                            rhs=xt_sb[:, j * MBLK:j * MBLK + mw],
                            start=(f == 0), stop=(f == NF - 1))
                for j in range(nmb):
                    m = mp + j
                    mw = min(MBLK, NPC - m * MBLK)
                    zt_sb = wpool.tile([H, MBLK], F32, tag="zt_sb")
                    nc.vector.tensor_copy(zt_sb[:, :mw], zt_list[j][:, :mw])
                    for k in range(mw // P):
                        t = m * (MBLK // P) + k
                        tr_ps = pspool.tile([P, H], F32, tag="pss")
                        nc.tensor.transpose(
                            tr_ps[:], zt_sb[:, k * P:(k + 1) * P],
                            ident[:H, :H])
                        nc.vector.tensor_scalar(
                            out=z1loc[:, t * H:(t + 1) * H], in0=tr_ps[:],
                            scalar1=dinv1_sb[:, t:t + 1], scalar2=None,
                            op0=ALU.mult)
                        nc.sync.dma_start(z1b[t * P:(t + 1) * P, :],
                                          z1loc[:, t * H:(t + 1) * H])

            nc.leave_named_scope("p1", sc, False)

            # ================= P2: AllGather z1' (direct to Local) =======
            sc = nc.enter_named_scope("ag1", False)[0]
            nc.gpsimd.collective_compute(
                "AllGather", ALU.bypass, replica_groups=rg,
                ins=[z1b.opt()], outs=[z1g[:, :]])
            nc.leave_named_scope("ag1", sc, False)

            # ================= P3: conv1 agg + elu =======================
            sc = nc.enter_named_scope("p3", False)[0]
            def p3_out(t, agg):
                if not skip_b1:
                    nc.vector.tensor_tensor(out=agg[:], in0=agg[:],
                                            in1=b1_sb[:], op=ALU.add)
                udst = u_loc[:, t * 2 * H: t * 2 * H + H]
                elu_into(udst, agg[:], wpool, H)
                nc.sync.dma_start(ub[t * P:(t + 1) * P, :], udst)
            agg_layer(z1g, p3_out, z1loc, tap=debug_taps)

            # extract local winner u-rows into compact wb [WMAX, H]
            wloc_sb = gpool.tile([P, (WMAX // P) * H], F32, tag="wloc")
            nc.gpsimd.dma_gather(
                out_ap=wloc_sb[:].rearrange("p (c h) -> p c h", h=H),
                in_ap=ub[0:NPC1, :], idxs_ap=wext_sb[:, :],
                num_idxs=WMAX, num_idxs_reg=WMAX, elem_size=H,
                single_packet=False, queue_num=3)
            nc.sync.dma_start(
                wb[0:WMAX, :].rearrange("(c p) h -> p c h", p=P),
                wloc_sb[:].rearrange("p (c h) -> p c h", h=H))
            nc.leave_named_scope("p3", sc, False)

            # ================= P4: AllGather winners (compact) ===========
            sc = nc.enter_named_scope("ag2", False)[0]
            nc.gpsimd.collective_compute(
                "AllGather", ALU.bypass, replica_groups=rg,
                ins=[wb.opt()], outs=[wg[:, :]])
            nc.leave_named_scope("ag2", sc, False)

            # ================= P5: x1 gather + z2 ========================
            sc = nc.enter_named_scope("p5", False)[0]
            xga = bpool.tile([P, NT * H], F32)
            nidx = NT * P
            nc.gpsimd.dma_gather(
                out_ap=xga[:].rearrange("p (c h) -> p c h", h=H),
                in_ap=wg[0:C * WMAX, :], idxs_ap=x1wc_sb[:, :],
                num_idxs=nidx, num_idxs_reg=nidx, elem_size=H,
                single_packet=False, queue_num=0)
            nc.vector.tensor_copy(
                u_loc[:].rearrange("p (t h) -> p t h", t=2 * NT)[:, 1::2, :],
                xga[:].rearrange("p (t h) -> p t h", t=NT))
            for t in range(NT):
                hT_ps = pspool.tile([P, P], F32, tag="pss")
                nc.tensor.transpose(
                    hT_ps[:], u_loc[:, t * 2 * H:(t + 1) * 2 * H], ident[:])
                hT_sb = wpool.tile([P, P], F32, tag="hT_sb")
                nc.vector.tensor_copy(hT_sb[:], hT_ps[:])
                z2_ps = pspool.tile([P, H], F32, tag="pss")
                nc.tensor.matmul(z2_ps[:], lhsT=hT_sb[:], rhs=w2_sb[:],
                                 start=True, stop=True)
                nc.vector.tensor_scalar(
                    out=z2loc[:, t * H:(t + 1) * H], in0=z2_ps[:],
                    scalar1=dinv_sb[:, t:t + 1], scalar2=None, op0=ALU.mult)
                nc.sync.dma_start(z2b[t * P:(t + 1) * P, :],
                                  z2loc[:, t * H:(t + 1) * H])

            nc.leave_named_scope("p5", sc, False)

            # ================= P6: AllGather z2' (direct to Local) =======
            sc = nc.enter_named_scope("ag3", False)[0]
            nc.gpsimd.collective_compute(
                "AllGather", ALU.bypass, replica_groups=rg,
                ins=[z2b.opt()], outs=[z2g[:, :]])
            nc.leave_named_scope("ag3", sc, False)

            # ================= P7: conv2 agg + head ======================
            sc = nc.enter_named_scope("p7", False)[0]
            vbuf = bpool.tile([P, NT], F32)

            def p7_out(t, agg):
                if not skip_b2:
                    nc.vector.tensor_tensor(out=agg[:], in0=agg[:],
                                            in1=b2_sb[:], op=ALU.add)
                e2 = wpool.tile([P, H], F32, tag="e2")
                elu_into(e2[:], agg[:], wpool, H)
                nc.vector.tensor_tensor(out=e2[:], in0=e2[:], in1=fcw_sb[:],
                                        op=ALU.mult)
                nc.vector.tensor_reduce(out=vbuf[:, t:t + 1], in_=e2[:],
                                        op=ALU.add, axis=AX.X)
            agg_layer(z2g, p7_out, z2loc)
            nc.leave_named_scope("p7", sc, False)

            sc = nc.enter_named_scope("head", False)[0]
            nc.vector.tensor_tensor(out=vbuf[:], in0=vbuf[:], in1=keep_sb[:],
                                    op=ALU.mult)
            nc.vector.tensor_tensor(out=vbuf[:], in0=vbuf[:], in1=mneg_sb[:],
                                    op=ALU.add)
            es = bpool.tile([P, NT], F32)
            acc = wpool.tile([P, 1], F32, tag="acc")
            nc.scalar.activation(es[:], vbuf[:], ACTF.Exp,
                                 bias=neg48_sb[:], scale=1.0,
                                 accum_out=acc[:])
            s_ps = pspool.tile([1, 1], F32, tag="pss")
            nc.tensor.matmul(s_ps[:], lhsT=acc[:], rhs=ones_sb[:],
                             start=True, stop=True)
            s_sb = wpool.tile([1, 1], F32, tag="s_sb")
            nc.vector.tensor_copy(s_sb[:], s_ps[:])
            nc.sync.dma_start(sj_in[:], s_sb[:])
            nc.gpsimd.collective_compute(
                "AllReduce", ALU.add, replica_groups=rg,
                ins=[sj_in.opt()], outs=[sj_out[:, :]])
            s2_sb = wpool.tile([1, 1], F32, tag="s2_sb")
            nc.sync.dma_start(s2_sb[:], sj_out[:, :])
            lnS = wpool.tile([1, 1], F32, tag="lnS")
            nc.scalar.activation(lnS[:], s2_sb[:], ACTF.Ln)
            b_ps = pspool.tile([P, 1], F32, tag="pss")
            nc.tensor.matmul(b_ps[:], lhsT=ones_row[:], rhs=lnS[:],
                             start=True, stop=True)
            bias_sb = wpool.tile([P, 1], F32, tag="bias_sb")
            nc.vector.tensor_scalar(out=bias_sb[:], in0=b_ps[:],
                                    scalar1=-1.0, scalar2=-SOFTMAX_SHIFT,
                                    op0=ALU.mult, op1=ALU.add)
            y_sb = bpool.tile([P, NT], F32)
            nc.vector.tensor_tensor(out=y_sb[:], in0=vbuf[:],
                                    in1=bias_sb[:].to_broadcast([P, NT]),
                                    op=ALU.add)
            nc.sync.dma_start(y_d.ap()[:], y_sb[:])
            nc.leave_named_scope("head", sc, False)
            if debug_taps:
                nc.sync.dma_start(z1dbg_d.ap()[:], z1b[:])
                nc.sync.dma_start(udbg_d.ap()[:], ub[:])
                nc.sync.dma_start(z2dbg_d.ap()[:], z2b[:])
                nc.sync.dma_start(vdbg_d.ap()[:], vbuf[:])

    nc.compile()
    return nc


# ---------------------------------------------------------------------------
# Full flow
# ---------------------------------------------------------------------------

def run(x, edge_index, all_edge_index, s_mapping_index, e_mask,
        conv1_w, conv1_b, conv2_w, conv2_b, fc_w, fc_b,
        C=8, mm_dtype=F32, trace=False, nc_cache=None, debug_taps=False,
        **rbk_kwargs):
    tabs, meta = host_prep(
        x, edge_index, all_edge_index, s_mapping_index, e_mask, C)
    w1, w2, b1r, b2r, fcwr, fcbr = host_prep_weights(
        conv1_w, conv1_b, conv2_w, conv2_b, fc_w, fc_b, meta)
    fcb_val = np.float32(np.asarray(fc_b).reshape(-1)[0])
    for c in range(C):
        tabs['mneg'][c] = (tabs['mneg'][c]
                           + fcb_val * tabs['keep'][c]).astype(np.float32)
    skip_b1 = bool(np.all(np.asarray(conv1_b) == 0))
    skip_b2 = bool(np.all(np.asarray(conv2_b) == 0))

    if nc_cache is not None and 'nc' in nc_cache:
        nc = nc_cache['nc']
    else:
        nc = build_kernel(meta, mm_dtype=mm_dtype, debug_taps=debug_taps,
                          skip_b1=skip_b1, skip_b2=skip_b2)
        if nc_cache is not None:
            nc_cache['nc'] = nc

    if mm_dtype == mybir.dt.bfloat16:
        import ml_dtypes
        w1 = w1.astype(ml_dtypes.bfloat16)
        tabs['xts'] = [xt.astype(ml_dtypes.bfloat16) for xt in tabs['xts']]
    elif mm_dtype == mybir.dt.float8e4:
        import ml_dtypes
        w1 = (w1 * 64.0).astype(ml_dtypes.float8_e4m3)
        tabs['xts'] = [xt.astype(ml_dtypes.float8_e4m3) for xt in tabs['xts']]

    in_maps = []
    for c in range(C):
        in_maps.append(dict(
            xt=tabs['xts'][c], w1=w1, w2=w2, b1r=b1r, b2r=b2r, fcwr=fcwr,
            fcbr=fcbr, itbl_lo=tabs['itbl_lo'][c], itbl_hi=tabs['itbl_hi'][c],
            wext=tabs['wext'][c], x1wc=tabs['x1wc'][c],
            degt=tabs['degt'][c], keep=tabs['keep'][c], mneg=tabs['mneg'][c]))
    res = bass_utils.run_bass_kernel_spmd(
        nc, in_maps, core_ids=list(range(C)), trace=trace, **rbk_kwargs)

    N = meta['N']
    n_per = meta['n_per']
    out = np.empty((N, 1), dtype=np.float32)
    for c in range(C):
        yc = res.results[c]['y']
        flat = yc.T.reshape(-1)
        out[meta['lperm'][c], 0] = flat[:n_per]
    return out, res, meta


# ---------------------------------------------------------------------------
# Harness entry point
# ---------------------------------------------------------------------------

_NC_CACHE = {}


def kernel(**inputs):
    """Full (unsharded) inputs -> full [N, 1] float32 output."""
    out, _res, _meta = run(
        x=np.asarray(inputs['x'], dtype=np.float32),
        edge_index=np.asarray(inputs['edge_index']),
        all_edge_index=np.asarray(inputs['all_edge_index']),
        s_mapping_index=np.asarray(inputs['s_mapping_index']),
        e_mask=np.asarray(inputs['e_mask']),
        conv1_w=np.asarray(inputs['conv1_w'], dtype=np.float32),
        conv1_b=np.asarray(inputs['conv1_b'], dtype=np.float32),
        conv2_w=np.asarray(inputs['conv2_w'], dtype=np.float32),
        conv2_b=np.asarray(inputs['conv2_b'], dtype=np.float32),
        fc_w=np.asarray(inputs['fc_w'], dtype=np.float32),
        fc_b=np.asarray(inputs['fc_b'], dtype=np.float32),
        C=8, mm_dtype=mybir.dt.float8e4, trace=False, nc_cache=_NC_CACHE)
    return out



# revision 17
# speedup vs baseline: 2.0192x; 2.0192x over previous
"""GCN EndNodeSelector Bass kernel for TRN2, 8-core SPMD — v2.

Architecture (per core, nodes row-sharded, identity permutation):
  P1: z1 = x @ W1 (PE, fp8 xT streamed, W1 pre-scaled x64), z1' = dinv1*z1
      evacuated from PSUM via ACT copy-scale (scalar engine). z1' cast to
      bf16 into a 256B-padded-row staging tile, one bulk DMA -> z1b.
  AG1: AllGather z1b -> z1g (local DRAM table, [C*NPC1, 128] bf16,
      only cols 0:64 meaningful). Collective issued from the SCALAR queue
      so it does not serialize behind gather preps on gpsimd.
  P3: conv1 aggregation: unpadded per-edge dma_gather (prepare_only +
      trigger_dma, 4 SWDGE queues, lo/hi table windows for int16 range),
      segmented reduction on the TENSOR engine via host-built 0/1 segment
      matrices S (fp8) accumulated in PSUM; dst-side dinv scale + PSUM
      evacuation on ACT. Self-loop term added in one batched DVE op
      (z1loc*dinvx); ELU batched (4 DVE ops on [128, NT*64]).
  P4/P5: winner-table AllGather (compact) + x1 gather; z2 = [u|x1u] @ W2
      per tile on PE with ACT evacuations.
  AG3 + P7: conv2 aggregation with the SAME edge structure: idx tables and
      S matrices are shared with conv1; only the source table differs
      (z2g). Head: fc + mask + global log_softmax via AllReduce.

Vector engine is kept out of the gather windows entirely (it shares an
SBUF port with the Q7 SWDGE descriptor generator; concurrent use stalls
both). Descriptor emission is the critical resource: ~4-7 ns/descriptor,
serialized on the gpsimd engine.
"""
import sys
import numpy as np

sys.path.insert(0, '/opt/trn_rl_repo')

import concourse.bass as bass
import concourse.bacc as bacc
import concourse.tile as tile
from concourse import mybir
from concourse import bass_utils
from concourse.masks import make_identity

F32 = mybir.dt.float32
BF16 = mybir.dt.bfloat16
FP8 = mybir.dt.float8e4
I16 = mybir.dt.int16
AX = mybir.AxisListType
ALU = mybir.AluOpType
ACTF = mybir.ActivationFunctionType

P = 128
H = 64
ROWB = 2 * H                 # 256B padded row: 128 bf16 (cols 0:64 real)
BIG_NEG = -1e9
SOFTMAX_SHIFT = 48.0
CHUNK_BLOCKS = 12            # gather chunk = 12 blocks of 128 slots
S_CHUNK_BLOCKS = 24          # S stream chunk (consumption order)


def _wrap_idx(flat):
    """dma_gather index layout: [128, ceil(n/16)] int16, list wrapped into 16
    partitions (i -> [i%16, i//16]) and replicated across the 8 Q7 groups."""
    flat = np.asarray(flat, dtype=np.int64)
    n = flat.size
    s = (n + 15) // 16
    pad = np.full(s * 16, 0, dtype=np.int64)
    pad[:n] = flat
    assert pad.max() < 32768
    arr = pad.reshape(s, 16).T.astype(np.int16)     # [16, s]
    return np.tile(arr, (8, 1))                      # [128, s]


# ---------------------------------------------------------------------------
# Host preprocessing
# ---------------------------------------------------------------------------

def host_prep(x, edge_index, all_edge_index, s_mapping_index, e_mask, C):
    N, F = x.shape
    n_per = N // C
    NPC = ((n_per + P - 1) // P) * P
    NT = NPC // P
    NPC1 = NPC + 1                       # +1 zero row per core chunk (ub only)
    ZLOC = NPC
    FP = ((F + P - 1) // P) * P
    C_LO = min(C - 1, 32767 // NPC1)
    assert C_LO >= 1 and (C - C_LO) * NPC1 <= 32768
    NLO = C_LO * NPC1

    src = np.asarray(edge_index[0], dtype=np.int64)
    dst = np.asarray(edge_index[1], dtype=np.int64)
    deg = np.bincount(dst, minlength=N).astype(np.float64) + 1.0
    dinv_all = (1.0 / np.sqrt(deg)).astype(np.float32)

    # table row of node n (identity permutation): c*NPC1 + local
    core_of = src // n_per
    row_of_src = core_of * NPC1 + (src % n_per)

    es = np.argsort(dst, kind='stable')
    dst_s = dst[es]
    src_row_s = row_of_src[es]
    starts = np.searchsorted(dst_s, np.arange(N))
    ends = np.searchsorted(dst_s, np.arange(N) + 1)

    # Per (core, tile, window): edge slot lists, padded per-tile to 128-mult,
    # block counts made UNIFORM across cores (SPMD single kernel).
    # slot arrays: idx (window-local row) + dst_local (or -1 for pad)
    tile_lists = [[[None, None] for _ in range(NT)] for _ in range(C)]
    nblk = np.zeros((NT, 2), dtype=np.int64)   # max over cores
    for c in range(C):
        for t in range(NT):
            lo_idx, lo_d, hi_idx, hi_d = [], [], [], []
            base = c * n_per + t * P
            nreal = min(P, n_per - t * P)
            for pp in range(nreal):
                v = base + pp
                rows = src_row_s[starts[v]:ends[v]]
                lo = rows[rows < NLO]
                hi = rows[rows >= NLO] - NLO
                lo_idx.extend(lo.tolist())
                lo_d.extend([pp] * lo.size)
                hi_idx.extend(hi.tolist())
                hi_d.extend([pp] * hi.size)
            tile_lists[c][t][0] = (np.asarray(lo_idx, np.int64),
                                   np.asarray(lo_d, np.int64))
            tile_lists[c][t][1] = (np.asarray(hi_idx, np.int64),
                                   np.asarray(hi_d, np.int64))
            nblk[t, 0] = max(nblk[t, 0], (len(lo_idx) + P - 1) // P)
            nblk[t, 1] = max(nblk[t, 1], (len(hi_idx) + P - 1) // P)
    nblk = np.maximum(nblk, 1)           # >=1 block per (tile, window)

    NBL_LO = int(nblk[:, 0].sum())
    NBL_HI = int(nblk[:, 1].sum())
    NBL = NBL_LO + NBL_HI

    # consumption order: per tile, lo blocks then hi blocks
    # block records: (tile, window, idx_in_window_seq)
    blocks = []                          # (t, w)
    for t in range(NT):
        blocks += [(t, 0)] * int(nblk[t, 0])
        blocks += [(t, 1)] * int(nblk[t, 1])

    # per-window block order (the gather stream order): lo blocks in
    # consumption order among themselves, likewise hi.
    # chunk structure per window:
    def chunkify(nb):
        out = []
        b = 0
        while b < nb:
            out.append(min(CHUNK_BLOCKS, nb - b))
            b += CHUNK_BLOCKS
        return out
    chunks_lo = chunkify(NBL_LO)
    chunks_hi = chunkify(NBL_HI)

    # Build per-core flat slot arrays per window (block-major, consumption
    # order within window), and S host array in consumption order.
    slot_idx = np.zeros((C, 2, max(NBL_LO, NBL_HI) * P), dtype=np.int64)
    S_host = np.zeros((C, P, NBL * P), dtype=np.float32)
    # map: consumption block index -> (window, seq within window)
    wseq = [0, 0]
    blk_map = []
    for (t, w) in blocks:
        blk_map.append((t, w, wseq[w]))
        wseq[w] += 1
    for c in range(C):
        fill = [0, 0]
        for bi, (t, w, sq) in enumerate(blk_map):
            idxs, ds = tile_lists[c][t][w]
            # which slice of this tile's list goes in this block?
            # blocks of tile t window w are consecutive in seq order
            pass
        # simpler: iterate tiles directly
        seq_base = [0, 0]
        cons_bi = 0
        for t in range(NT):
            for w in (0, 1):
                idxs, ds = tile_lists[c][t][w]
                nb = int(nblk[t, w])
                nsl = nb * P
                ii = np.zeros(nsl, dtype=np.int64)
                dd = np.full(nsl, -1, dtype=np.int64)
                ii[:idxs.size] = idxs
                dd[:ds.size] = ds
                sb = seq_base[w]
                slot_idx[c, w, sb * P:sb * P + nsl] = ii
                # S: consumption block indices for this tile/window
                for j in range(nb):
                    gb = cons_bi + j          # consumption order block id
                    d = dd[j * P:(j + 1) * P]
                    k = np.nonzero(d >= 0)[0]
                    S_host[c, k, gb * P + d[k]] = 1.0
                seq_base[w] += nb
                cons_bi += nb
    assert cons_bi == NBL

    # idx tables (wrapped int16) per window, concatenated chunk tables
    def build_itbl(w, nbl_w):
        per_core = []
        coffs = [0]
        for c in range(C):
            parts = []
            b = 0
            for nb in (chunks_lo if w == 0 else chunks_hi):
                flat = slot_idx[c, w, b * P:(b + nb) * P]
                parts.append(_wrap_idx(flat))
                b += nb
            cat = np.concatenate(parts, axis=1)
            per_core.append(cat)
            if c == 0:
                coffs = np.cumsum([0] + [q.shape[1] for q in parts])
        return np.stack(per_core), list(coffs)
    itbl_lo, lo_coffs = build_itbl(0, NBL_LO)
    itbl_hi, hi_coffs = build_itbl(1, NBL_HI)

    # consumption plan: for each block (consumption order):
    #   (window w, chunk index within window cw, column within chunk)
    plan = []
    seq_in_w = [0, 0]
    for (t, w) in blocks:
        sq = seq_in_w[w]
        plan.append((t, w, sq // CHUNK_BLOCKS, sq % CHUNK_BLOCKS))
        seq_in_w[w] += 1

    # ---------------- x1 winner path (last-write-wins) ------------------
    map_vec = np.full(N, -1, dtype=np.int64)
    map_vec[np.asarray(s_mapping_index[1], dtype=np.int64)] = np.asarray(
        s_mapping_index[0], dtype=np.int64)
    gen = map_vec[np.asarray(all_edge_index[0], dtype=np.int64)]
    valid = gen >= 0
    tgt = np.asarray(all_edge_index[1], dtype=np.int64)[valid]
    genv = gen[valid]
    x1_src = np.full(N, -1, dtype=np.int64)
    if tgt.size:
        u_t, first_rev = np.unique(tgt[::-1], return_index=True)
        x1_src[u_t] = genv[tgt.size - 1 - first_rev]

    winners = np.unique(x1_src[x1_src >= 0])
    wcore = winners // n_per
    wloc = winners % n_per
    wlists = [np.sort(wloc[wcore == c]) for c in range(C)]
    maxlen = max((len(w) for w in wlists), default=0)
    WMAX = ((maxlen + 1 + 127) // 128) * 128
    assert C * WMAX < 32768
    pos_of = {}
    wext = np.empty((C, P, WMAX // 16), dtype=np.int16)
    for c in range(C):
        lst = wlists[c]
        for i, l in enumerate(lst):
            pos_of[(c, int(l))] = c * WMAX + i
        flat = np.full(WMAX, ZLOC, np.int64)
        flat[:len(lst)] = lst
        wext[c] = _wrap_idx(flat)[:, :WMAX // 16]

    DEAD = WMAX - 1                       # core0 pad slot => zeros
    x1wc = np.empty((C, P, 8 * NT), dtype=np.int16)
    for c in range(C):
        flat = np.full(NT * P, DEAD, np.int64)
        for t in range(NT):
            nreal = min(P, n_per - t * P)
            for pp in range(nreal):
                g = x1_src[c * n_per + t * P + pp]
                if g >= 0:
                    flat[t * P + pp] = pos_of[(g // n_per, int(g % n_per))]
        assert flat.max() < 32768
        x1wc[c] = _wrap_idx(flat)

    # ---------------- per-node tiles ------------------------------------
    maskf = np.asarray(e_mask).reshape(-1).astype(bool)
    dinv = np.ones((C, P, NT), dtype=np.float32)
    keep = np.zeros((C, P, NT), dtype=np.float32)
    mneg = np.full((C, P, NT), np.float32(BIG_NEG), dtype=np.float32)
    for c in range(C):
        dp = np.ones(NPC, dtype=np.float32)
        dp[:n_per] = dinv_all[c * n_per:(c + 1) * n_per]
        kp = np.zeros(NPC, dtype=np.float32)
        kp[:n_per] = (~maskf[c * n_per:(c + 1) * n_per]).astype(np.float32)
        mp = np.full(NPC, np.float32(BIG_NEG), dtype=np.float32)
        mp[:n_per] = np.where(maskf[c * n_per:(c + 1) * n_per],
                              np.float32(BIG_NEG), np.float32(0.0))
        dinv[c] = dp.reshape(NT, P).T
        keep[c] = kp.reshape(NT, P).T
        mneg[c] = mp.reshape(NT, P).T
    dinvx = np.repeat(dinv.reshape(C, P, NT, 1), H, axis=3).reshape(
        C, P, NT * H)

    xts = []
    xf = np.asarray(x, dtype=np.float32)
    for c in range(C):
        xt = np.zeros((FP, NPC), dtype=np.float32)
        xt[:F, :n_per] = xf[c * n_per:(c + 1) * n_per].T
        xts.append(xt)

    meta = dict(N=N, F=F, C=C, n_per=n_per, NPC=NPC, NPC1=NPC1, NT=NT,
                FP=FP, C_LO=C_LO, NLO=NLO, WMAX=WMAX,
                NBL=NBL, NBL_LO=NBL_LO, NBL_HI=NBL_HI,
                chunks_lo=chunks_lo, chunks_hi=chunks_hi,
                lo_coffs=lo_coffs, hi_coffs=hi_coffs,
                plan=plan, nblk=nblk)
    return dict(xts=xts, itbl_lo=itbl_lo, itbl_hi=itbl_hi,
                wext=wext, x1wc=x1wc, S=S_host,
                dinv=dinv, dinvx=dinvx, keep=keep, mneg=mneg), meta


def host_prep_weights(conv1_w, conv1_b, conv2_w, conv2_b, fc_w, fc_b, meta):
    FP = meta['FP']
    F = meta['F']
    NT = meta['NT']
    w1 = np.zeros((FP, H), dtype=np.float32)
    w1[:F] = np.asarray(conv1_w, dtype=np.float32)
    w2 = np.asarray(conv2_w, dtype=np.float32)
    fcwx = np.tile(np.asarray(fc_w, np.float32).reshape(1, H), (P, NT)).copy()
    return w1, w2, fcwx


# ---------------------------------------------------------------------------
# Device kernel
# ---------------------------------------------------------------------------

def build_kernel(meta, s_dtype=FP8, debug_taps=False):
    C = meta['C']
    NPC = meta['NPC']
    NPC1 = meta['NPC1']
    NT = meta['NT']
    FP = meta['FP']
    C_LO = meta['C_LO']
    NLO = meta['NLO']
    WMAX = meta['WMAX']
    NBL = meta['NBL']
    chunks_lo = meta['chunks_lo']
    chunks_hi = meta['chunks_hi']
    lo_coffs = meta['lo_coffs']
    hi_coffs = meta['hi_coffs']
    plan = meta['plan']
    NF = FP // P
    MBLK = 512
    NMB = 3                              # m-blocks per PSUM pass
    n_mblk = (NPC + MBLK - 1) // MBLK
    NHI = (C - C_LO) * NPC1
    SUMW_LO = sum(((nb * P) + 15) // 16 for nb in chunks_lo)
    SUMW_HI = sum(((nb * P) + 15) // 16 for nb in chunks_hi)

    nc = bacc.Bacc("TRN2", target_bir_lowering=False, debug=False,
                   num_devices=C, num_swdge_queues=4,
                   dynamic_dma_scratch_size=32768)

    xt_d = nc.dram_tensor("xt", [FP, NPC], FP8, kind="ExternalInput")
    w1_d = nc.dram_tensor("w1", [FP, H], FP8, kind="ExternalInput")
    w2_d = nc.dram_tensor("w2", [2 * H, H], F32, kind="ExternalInput")
    fcwx_d = nc.dram_tensor("fcwx", [P, NT * H], F32, kind="ExternalInput")
    dinv_d = nc.dram_tensor("dinv", [P, NT], F32, kind="ExternalInput")
    dinv1_d = nc.dram_tensor("dinv1", [P, NT], F32, kind="ExternalInput")
    dinvx_d = nc.dram_tensor("dinvx", [P, NT * H], F32, kind="ExternalInput")
    keep_d = nc.dram_tensor("keep", [P, NT], F32, kind="ExternalInput")
    mneg_d = nc.dram_tensor("mneg", [P, NT], F32, kind="ExternalInput")
    ilo_d = nc.dram_tensor("itbl_lo", [P, SUMW_LO], I16, kind="ExternalInput")
    ihi_d = nc.dram_tensor("itbl_hi", [P, SUMW_HI], I16, kind="ExternalInput")
    wext_d = nc.dram_tensor("wext", [P, WMAX // 16], I16, kind="ExternalInput")
    x1wc_d = nc.dram_tensor("x1wc", [P, 8 * NT], I16, kind="ExternalInput")
    s_d = nc.dram_tensor("S", [P, NBL * P], s_dtype, kind="ExternalInput")
    y_d = nc.dram_tensor("y", [P, NT], F32, kind="ExternalOutput")
    if debug_taps:
        z1dbg_d = nc.dram_tensor("z1dbg", [NPC1, ROWB], BF16,
                                 kind="ExternalOutput")
        udbg_d = nc.dram_tensor("udbg", [NPC1, ROWB], BF16,
                                kind="ExternalOutput")
        z2dbg_d = nc.dram_tensor("z2dbg", [NPC1, ROWB], BF16,
                                 kind="ExternalOutput")
        aggdbg_d = nc.dram_tensor("aggdbg", [P, NT * H], F32,
                                  kind="ExternalOutput")
        vdbg_d = nc.dram_tensor("vdbg", [P, NT], F32, kind="ExternalOutput")

    rg = [list(range(C))]

    with tile.TileContext(nc) as tc:
        with tc.tile_pool(name="const", bufs=1) as cpool, \
             tc.tile_pool(name="xin", bufs=4) as xpool, \
             tc.tile_pool(name="work", bufs=2) as wpool, \
             tc.tile_pool(name="sstr", bufs=2) as spool, \
             tc.tile_pool(name="g0", bufs=2) as gp0, \
             tc.tile_pool(name="g1", bufs=2) as gp1, \
             tc.tile_pool(name="g2", bufs=2) as gp2, \
             tc.tile_pool(name="g3", bufs=2) as gp3, \
             tc.tile_pool(name="big", bufs=1) as bpool, \
             tc.tile_pool(name="psz", bufs=1, space="PSUM") as pszpool, \
             tc.tile_pool(name="agg", bufs=2, space="PSUM") as aggpool, \
             tc.tile_pool(name="ps", bufs=3, space="PSUM") as pspool, \
             tc.tile_pool(name="dram", bufs=1, space="DRAM") as dpool:

            gpools = [gp0, gp1, gp2, gp3]

            z1b = dpool.tile([NPC1, ROWB], BF16)
            ub = dpool.tile([NPC1, ROWB], BF16)
            z2b = dpool.tile([NPC1, ROWB], BF16)
            wb = dpool.tile([WMAX, ROWB], BF16)
            z1g = nc.dram_tensor("z1g_loc", [C * NPC1, ROWB], BF16,
                                 kind="Internal", addr_space="Local").ap()
            wg = nc.dram_tensor("wg_loc", [C * WMAX, ROWB], BF16,
                                kind="Internal", addr_space="Local").ap()
            z2g = nc.dram_tensor("z2g_loc", [C * NPC1, ROWB], BF16,
                                 kind="Internal", addr_space="Local").ap()
            sj_in = dpool.tile([1, 1], F32)
            ash = "Shared" if C > 4 else "Local"
            sj_out = nc.dram_tensor("sj_sh", [1, 1], F32,
                                    kind="Internal", addr_space=ash).ap()

            # ---------------- constants -----------------------------------
            ident = cpool.tile([P, P], F32)
            make_identity(nc, ident[:])
            w1_sb = cpool.tile([P, NF * H], FP8)
            nc.sync.dma_start(
                w1_sb[:], w1_d.ap().rearrange("(a p) h -> p a h", p=P))
            w2_sb = cpool.tile([P, H], F32)
            nc.sync.dma_start(w2_sb[:], w2_d.ap())
            fcwx_sb = cpool.tile([P, NT * H], F32)
            nc.sync.dma_start(fcwx_sb[:], fcwx_d.ap())
            dinv_sb = cpool.tile([P, NT], F32)
            nc.sync.dma_start(dinv_sb[:], dinv_d.ap())
            dinv1_sb = cpool.tile([P, NT], F32)
            nc.sync.dma_start(dinv1_sb[:], dinv1_d.ap())
            dinvx_sb = cpool.tile([P, NT * H], F32)
            nc.sync.dma_start(dinvx_sb[:], dinvx_d.ap())
            keep_sb = cpool.tile([P, NT], F32)
            nc.sync.dma_start(keep_sb[:], keep_d.ap())
            mneg_sb = cpool.tile([P, NT], F32)
            nc.sync.dma_start(mneg_sb[:], mneg_d.ap())
            ilo_sb = cpool.tile([P, SUMW_LO], I16)
            nc.sync.dma_start(ilo_sb[:], ilo_d.ap())
            ihi_sb = cpool.tile([P, SUMW_HI], I16)
            nc.sync.dma_start(ihi_sb[:], ihi_d.ap())
            wext_sb = cpool.tile([P, WMAX // 16], I16)
            nc.sync.dma_start(wext_sb[:], wext_d.ap())
            x1wc_sb = cpool.tile([P, 8 * NT], I16)
            nc.sync.dma_start(x1wc_sb[:], x1wc_d.ap())

            ones_sb = cpool.tile([P, 1], F32)
            nc.vector.memset(ones_sb[:], 1.0)
            ones_row = cpool.tile([1, P], F32)
            nc.vector.memset(ones_row[:], 1.0)
            neg48_sb = cpool.tile([P, 1], F32)
            nc.vector.memset(neg48_sb[:], -SOFTMAX_SHIFT)
            zrow_sb = cpool.tile([1, ROWB], BF16)
            nc.vector.memset(zrow_sb[:], 0.0)
            nc.sync.dma_start(ub[NPC:NPC + 1, :], zrow_sb[:])

            # staging tile (bf16, zero pad columns persist across reuses)
            stage = bpool.tile([P, NT * ROWB], BF16)
            nc.vector.memset(stage[:], 0.0)

            def stage3():
                return stage[:].rearrange("p (t r) -> p t r", r=ROWB)

            u_loc = bpool.tile([P, NT * 2 * H], F32)   # [u | x1u] interleaved
            z1loc = bpool.tile([P, NT * H], F32)       # z1' then z2' (reused)
            vz = bpool.tile([P, NT * H], F32)          # conv2 agg (+tmp)
            xga = bpool.tile([P, NT * ROWB], BF16)     # x1 gather out

            # ============ gather machinery ================================
            # chunk schedule (consumption order per window):
            #   lo chunks -> queues 0/1 alternating; hi -> 2/3
            def chunk_queue(w, cw):
                return (0 if w == 0 else 2) + (cw % 2)

            def chunk_gbuf(w, cw):
                q = chunk_queue(w, cw)
                return q, gpools[q].tile(
                    [P, CHUNK_BLOCKS * ROWB], BF16, tag=f"gb{q}")

            # Gather emission for one layer.  pre_fn (the AllGather
            # producing src_g) MUST be emitted before the first gather: Tile
            # binds the table-read dep at emission time, so a gather emitted
            # before the collective would read the stale table.  Plain
            # (self-triggered) gathers: emission ~1-3us/chunk on Q7, drains
            # async on the 4 SWDGE queues, buffer-reuse WAR paces the
            # pipeline (4 queues x 2 buffers in flight).
            def emit_layer_gathers(src_g, pre_fn=None):
                if pre_fn is not None:
                    pre_fn()
                first_pos = {}
                for bi, (t, w, cw, col) in enumerate(plan):
                    if (w, cw) not in first_pos:
                        first_pos[(w, cw)] = bi
                stream = sorted(first_pos, key=lambda k: first_pos[k])
                gtiles = {}
                for (w, cw) in stream:
                    q = chunk_queue(w, cw)
                    gt = gpools[q].tile(
                        [P, CHUNK_BLOCKS * ROWB], BF16, tag=f"gb{q}")
                    gtiles[(w, cw)] = gt
                    nb = (chunks_lo if w == 0 else chunks_hi)[cw]
                    nidx = nb * P
                    coffs = lo_coffs if w == 0 else hi_coffs
                    isb = ilo_sb if w == 0 else ihi_sb
                    win = (src_g[0:NLO, :] if w == 0
                           else src_g[NLO:NLO + NHI, :])
                    nc.gpsimd.dma_gather(
                        out_ap=gt[:, :nb * ROWB].rearrange(
                            "p (c r) -> p c r", r=ROWB),
                        in_ap=win,
                        idxs_ap=isb[:, coffs[cw]:coffs[cw + 1]],
                        num_idxs=nidx, num_idxs_reg=nidx, elem_size=ROWB,
                        single_packet=False, queue_num=q)
                return gtiles

            # PE consumption of one layer: segmented matmul into PSUM, then
            # ACT copy-scale into dst[:, t] slices
            def consume_layer(gtiles, out_cb):
                s_off = 0
                s_tile = None
                s_cols = 0
                for bi, (t, w, cw, col) in enumerate(plan):
                    if bi % S_CHUNK_BLOCKS == 0:
                        nbs = min(S_CHUNK_BLOCKS, NBL - bi)
                        s_tile = spool.tile([P, S_CHUNK_BLOCKS * P], s_dtype,
                                            tag="s")
                        nc.sync.dma_start(
                            s_tile[:, :nbs * P],
                            s_d.ap()[:, bi * P:(bi + nbs) * P])
                        s_off = bi
                    if bi == 0 or plan[bi - 1][0] != t:
                        ps = aggpool.tile([P, H], F32, tag="agg")
                        first = True
                    gt = gtiles[(w, cw)]
                    last = (bi == NBL - 1) or (plan[bi + 1][0] != t)
                    sc = bi - s_off
                    nc.tensor.matmul(
                        ps[:],
                        lhsT=s_tile[:, sc * P:(sc + 1) * P],
                        rhs=gt[:].rearrange("p (c r) -> p c r",
                                            r=ROWB)[:, col, 0:H],
                        start=first, stop=last)
                    first = False
                    if last:
                        out_cb(t, ps)

            # ================= P1: z1 = x @ W1 ===========================
            sc = nc.enter_named_scope("p1", False)[0]
            for mp in range(0, n_mblk, NMB):
                nmb = min(NMB, n_mblk - mp)
                w0 = min(MBLK * nmb, NPC - mp * MBLK)
                zt_list = [pszpool.tile([H, MBLK], F32, tag=f"zt{j}",
                                        name=f"ztp{j}")
                           for j in range(nmb)]
                for f in range(NF):
                    xt_sb = xpool.tile([P, NMB * MBLK], FP8, tag="xt")
                    eng = nc.sync if f % 2 == 0 else nc.scalar
                    eng.dma_start(
                        xt_sb[:, :w0],
                        xt_d.ap()[f * P:(f + 1) * P,
                                  mp * MBLK:mp * MBLK + w0])
                    for j in range(nmb):
                        mw = min(MBLK, NPC - (mp + j) * MBLK)
                        nc.tensor.matmul(
                            zt_list[j][:, :mw],
                            lhsT=w1_sb[:, f * H:(f + 1) * H],
                            rhs=xt_sb[:, j * MBLK:j * MBLK + mw],
                            start=(f == 0), stop=(f == NF - 1))
                for j in range(nmb):
                    m = mp + j
                    mw = min(MBLK, NPC - m * MBLK)
                    zt_sb = wpool.tile([H, MBLK], F32, tag="zt_sb")
                    nc.scalar.activation(zt_sb[:, :mw], zt_list[j][:, :mw],
                                         ACTF.Copy)
                    for k in range(mw // P):
                        t = m * (MBLK // P) + k
                        tr_ps = pspool.tile([P, P], F32, tag="pss")
                        nc.tensor.transpose(
                            tr_ps[:, :H], zt_sb[:, k * P:(k + 1) * P],
                            ident[:H, :H])
                        nc.scalar.activation(
                            z1loc[:, t * H:(t + 1) * H], tr_ps[:, :H],
                            ACTF.Copy, scale=dinv1_sb[:, t:t + 1])
            # batched bf16 cast into staging + one bulk DMA
            nc.vector.tensor_copy(
                stage3()[:, :, 0:H],
                z1loc[:].rearrange("p (t h) -> p t h", h=H))
            nc.sync.dma_start(
                z1b[0:NPC, :].rearrange("(t p) r -> p t r", p=P),
                stage3())
            nc.leave_named_scope("p1", sc, False)

            # ================= P3: conv1 agg ==============================
            sc = nc.enter_named_scope("p3", False)[0]

            def ag1_fn():
                nc.gpsimd.collective_compute(
                    "AllGather", ALU.bypass, replica_groups=rg,
                    ins=[z1b.opt()], outs=[z1g[:, :]])
            gtiles1 = emit_layer_gathers(z1g, pre_fn=ag1_fn)

            def p3_out(t, ps):
                nc.scalar.activation(
                    u_loc[:, t * 2 * H:t * 2 * H + H], ps[:],
                    ACTF.Copy, scale=dinv_sb[:, t:t + 1])
            consume_layer(gtiles1, p3_out)
            if debug_taps:
                nc.vector.tensor_copy(
                    vz[:].rearrange("p (t h) -> p t h", h=H),
                    u_loc[:].rearrange(
                        "p (t h) -> p t h", h=2 * H)[:, :, 0:H])
                nc.sync.dma_start(aggdbg_d.ap()[:], vz[:])

            # batched: self-loop add, ELU, bf16 cast, bulk DMA to ub
            def ue():
                return u_loc[:].rearrange(
                    "p (t h) -> p t h", h=2 * H)[:, :, 0:H]

            def mn3():
                return vz[:].rearrange("p (t h) -> p t h", h=H)
            nc.vector.tensor_tensor(out=vz[:], in0=z1loc[:], in1=dinvx_sb[:],
                                    op=ALU.mult)
            nc.vector.tensor_tensor(out=ue(), in0=ue(), in1=mn3(),
                                    op=ALU.add)
            # ELU in place on ue: mn=min(x,0); ex=exp(mn); x=max(x,0)+ex-1
            nc.vector.tensor_scalar(out=mn3(), in0=ue(), scalar1=0.0,
                                    scalar2=None, op0=ALU.min)
            nc.scalar.activation(mn3(), mn3(), ACTF.Exp)
            nc.vector.tensor_scalar(out=ue(), in0=ue(), scalar1=0.0,
                                    scalar2=None, op0=ALU.max)
            nc.vector.tensor_scalar(out=mn3(), in0=mn3(), scalar1=-1.0,
                                    scalar2=None, op0=ALU.add)
            nc.vector.tensor_tensor(out=ue(), in0=ue(), in1=mn3(),
                                    op=ALU.add)
            nc.vector.tensor_copy(stage3()[:, :, 0:H], ue())
            nc.sync.dma_start(
                ub[0:NPC, :].rearrange("(t p) r -> p t r", p=P),
                stage3())

            # winner extraction (compact table)
            wloc_sb = wpool.tile([P, (WMAX // P) * ROWB], BF16, tag="wloc")
            nc.gpsimd.dma_gather(
                out_ap=wloc_sb[:].rearrange("p (c r) -> p c r", r=ROWB),
                in_ap=ub[0:NPC1, :], idxs_ap=wext_sb[:, :],
                num_idxs=WMAX, num_idxs_reg=WMAX, elem_size=ROWB,
                single_packet=False, queue_num=0)
            nc.sync.dma_start(
                wb[0:WMAX, :].rearrange("(c p) r -> p c r", p=P),
                wloc_sb[:].rearrange("p (c r) -> p c r", r=ROWB))
            nc.leave_named_scope("p3", sc, False)

            # ================= AG2: winners ===============================
            sc = nc.enter_named_scope("ag2", False)[0]
            nc.gpsimd.collective_compute(
                "AllGather", ALU.bypass, replica_groups=rg,
                ins=[wb.opt()], outs=[wg[:, :]])
            nc.leave_named_scope("ag2", sc, False)

            # ================= P5: x1 gather + z2 ========================
            sc = nc.enter_named_scope("p5", False)[0]
            nidx = NT * P
            nc.gpsimd.dma_gather(
                out_ap=xga[:].rearrange("p (c r) -> p c r", r=ROWB),
                in_ap=wg[0:C * WMAX, :], idxs_ap=x1wc_sb[:, :],
                num_idxs=nidx, num_idxs_reg=nidx, elem_size=ROWB,
                single_packet=False, queue_num=1)
            nc.vector.tensor_copy(
                u_loc[:].rearrange("p (t h) -> p t h", h=2 * H)[:, :, H:],
                xga[:].rearrange("p (t r) -> p t r", r=ROWB)[:, :, 0:H])
            for t in range(NT):
                hT_ps = pspool.tile([P, P], F32, tag="pss")
                nc.tensor.transpose(
                    hT_ps[:], u_loc[:, t * 2 * H:(t + 1) * 2 * H], ident[:])
                hT_sb = wpool.tile([P, P], F32, tag="hT_sb")
                nc.scalar.activation(hT_sb[:], hT_ps[:], ACTF.Copy)
                z2_ps = pspool.tile([P, P], F32, tag="pss")
                nc.tensor.matmul(z2_ps[:, :H], lhsT=hT_sb[:], rhs=w2_sb[:],
                                 start=True, stop=True)
                nc.scalar.activation(
                    z1loc[:, t * H:(t + 1) * H], z2_ps[:, :H],
                    ACTF.Copy, scale=dinv_sb[:, t:t + 1])
            nc.vector.tensor_copy(
                stage3()[:, :, 0:H],
                z1loc[:].rearrange("p (t h) -> p t h", h=H))
            nc.sync.dma_start(
                z2b[0:NPC, :].rearrange("(t p) r -> p t r", p=P),
                stage3())
            nc.leave_named_scope("p5", sc, False)

            # ================= P7: conv2 agg + head ======================
            sc = nc.enter_named_scope("p7", False)[0]

            def ag3_fn():
                nc.gpsimd.collective_compute(
                    "AllGather", ALU.bypass, replica_groups=rg,
                    ins=[z2b.opt()], outs=[z2g[:, :]])
            gtiles2 = emit_layer_gathers(z2g, pre_fn=ag3_fn)

            def p7_out(t, ps):
                nc.scalar.activation(
                    vz[:, t * H:(t + 1) * H], ps[:],
                    ACTF.Copy, scale=dinv_sb[:, t:t + 1])
            consume_layer(gtiles2, p7_out)

            # batched: self add, ELU, fc reduce
            tmp = u_loc[:, 0:NT * H]       # u_loc no longer needed
            nc.vector.tensor_tensor(out=tmp, in0=z1loc[:], in1=dinvx_sb[:],
                                    op=ALU.mult)
            nc.vector.tensor_tensor(out=vz[:], in0=vz[:], in1=tmp,
                                    op=ALU.add)
            mn = u_loc[:, 0:NT * H]
            nc.vector.tensor_scalar(out=mn, in0=vz[:], scalar1=0.0,
                                    scalar2=None, op0=ALU.min)
            nc.scalar.activation(mn, mn, ACTF.Exp)
            nc.vector.tensor_scalar(out=vz[:], in0=vz[:], scalar1=0.0,
                                    scalar2=None, op0=ALU.max)
            nc.vector.tensor_scalar(out=mn, in0=mn, scalar1=-1.0,
                                    scalar2=None, op0=ALU.add)
            nc.vector.tensor_tensor(out=vz[:], in0=vz[:], in1=mn,
                                    op=ALU.add)
            nc.vector.tensor_tensor(out=vz[:], in0=vz[:], in1=fcwx_sb[:],
                                    op=ALU.mult)
            vbuf = wpool.tile([P, NT], F32, tag="vbuf")
            nc.vector.tensor_reduce(
                out=vbuf[:],
                in_=vz[:].rearrange("p (t h) -> p t h", h=H),
                op=ALU.add, axis=AX.X)
            nc.leave_named_scope("p7", sc, False)

            sc = nc.enter_named_scope("head", False)[0]
            nc.vector.tensor_tensor(out=vbuf[:], in0=vbuf[:], in1=keep_sb[:],
                                    op=ALU.mult)
            nc.vector.tensor_tensor(out=vbuf[:], in0=vbuf[:], in1=mneg_sb[:],
                                    op=ALU.add)
            es = wpool.tile([P, NT], F32, tag="es")
            acc = wpool.tile([P, 1], F32, tag="acc")
            nc.scalar.activation(es[:], vbuf[:], ACTF.Exp,
                                 bias=neg48_sb[:], scale=1.0,
                                 accum_out=acc[:])
            s_ps = pspool.tile([1, 1], F32, tag="pss")
            nc.tensor.matmul(s_ps[:], lhsT=acc[:], rhs=ones_sb[:],
                             start=True, stop=True)
            s_sb = wpool.tile([1, 1], F32, tag="s_sb")
            nc.vector.tensor_copy(s_sb[:], s_ps[:])
            nc.sync.dma_start(sj_in[:], s_sb[:])
            nc.gpsimd.collective_compute(
                "AllReduce", ALU.add, replica_groups=rg,
                ins=[sj_in.opt()], outs=[sj_out[:, :]])
            s2_sb = wpool.tile([1, 1], F32, tag="s2_sb")
            nc.sync.dma_start(s2_sb[:], sj_out[:, :])
            lnS = wpool.tile([1, 1], F32, tag="lnS")
            nc.scalar.activation(lnS[:], s2_sb[:], ACTF.Ln)
            b_ps = pspool.tile([P, 1], F32, tag="pss")
            nc.tensor.matmul(b_ps[:], lhsT=ones_row[:], rhs=lnS[:],
                             start=True, stop=True)
            bias_sb = wpool.tile([P, 1], F32, tag="bias_sb")
            nc.vector.tensor_scalar(out=bias_sb[:], in0=b_ps[:],
                                    scalar1=-1.0, scalar2=-SOFTMAX_SHIFT,
                                    op0=ALU.mult, op1=ALU.add)
            y_sb = wpool.tile([P, NT], F32, tag="y_sb")
            nc.vector.tensor_tensor(out=y_sb[:], in0=vbuf[:],
                                    in1=bias_sb[:].to_broadcast([P, NT]),
                                    op=ALU.add)
            nc.sync.dma_start(y_d.ap()[:], y_sb[:])
            if debug_taps:
                nc.sync.dma_start(z1dbg_d.ap()[:], z1b[:])
                nc.sync.dma_start(udbg_d.ap()[:], ub[:])
                nc.sync.dma_start(z2dbg_d.ap()[:], z2b[:])
                nc.sync.dma_start(vdbg_d.ap()[:], vbuf[:])
            nc.leave_named_scope("head", sc, False)

    nc.compile()
    return nc


# ---------------------------------------------------------------------------
# Full flow
# ---------------------------------------------------------------------------

def run(x, edge_index, all_edge_index, s_mapping_index, e_mask,
        conv1_w, conv1_b, conv2_w, conv2_b, fc_w, fc_b,
        C=8, trace=False, nc_cache=None, s_dtype=FP8, debug_taps=False,
        **rbk_kwargs):
    import ml_dtypes
    assert np.all(np.asarray(conv1_b) == 0) and np.all(
        np.asarray(conv2_b) == 0)
    tabs, meta = host_prep(
        x, edge_index, all_edge_index, s_mapping_index, e_mask, C)
    w1, w2, fcwx = host_prep_weights(
        conv1_w, conv1_b, conv2_w, conv2_b, fc_w, fc_b, meta)
    fcb_val = np.float32(np.asarray(fc_b).reshape(-1)[0])
    for c in range(C):
        tabs['mneg'][c] = (tabs['mneg'][c]
                           + fcb_val * tabs['keep'][c]).astype(np.float32)

    if nc_cache is not None and 'nc' in nc_cache:
        nc = nc_cache['nc']
    else:
        nc = build_kernel(meta, s_dtype=s_dtype, debug_taps=debug_taps)
        if nc_cache is not None:
            nc_cache['nc'] = nc

    w1_8 = (w1 * 64.0).astype(ml_dtypes.float8_e4m3)
    dinv1 = (tabs['dinv'] / 64.0).astype(np.float32)
    S8 = (tabs['S'].astype(ml_dtypes.float8_e4m3) if s_dtype == FP8
          else tabs['S'].astype(ml_dtypes.bfloat16))

    in_maps = []
    for c in range(C):
        in_maps.append(dict(
            xt=tabs['xts'][c].astype(ml_dtypes.float8_e4m3), w1=w1_8, w2=w2,
            fcwx=fcwx, dinv=tabs['dinv'][c], dinv1=dinv1[c],
            dinvx=tabs['dinvx'][c], keep=tabs['keep'][c],
            mneg=tabs['mneg'][c],
            itbl_lo=tabs['itbl_lo'][c], itbl_hi=tabs['itbl_hi'][c],
            wext=tabs['wext'][c], x1wc=tabs['x1wc'][c], S=S8[c]))
    res = bass_utils.run_bass_kernel_spmd(
        nc, in_maps, core_ids=list(range(C)), trace=trace, **rbk_kwargs)

    N = meta['N']
    n_per = meta['n_per']
    out = np.empty((N, 1), dtype=np.float32)
    for c in range(C):
        yc = res.results[c]['y']
        out[c * n_per:(c + 1) * n_per, 0] = yc.T.reshape(-1)[:n_per]
    return out, res, meta


# ---------------------------------------------------------------------------
# Harness entry point
# ---------------------------------------------------------------------------

_NC_CACHE = {}


def kernel(**inputs):
    """Full (unsharded) inputs -> full [N, 1] float32 output."""
    out, _res, _meta = run(
        x=np.asarray(inputs['x'], dtype=np.float32),
        edge_index=np.asarray(inputs['edge_index']),
        all_edge_index=np.asarray(inputs['all_edge_index']),
        s_mapping_index=np.asarray(inputs['s_mapping_index']),
        e_mask=np.asarray(inputs['e_mask']),
        conv1_w=np.asarray(inputs['conv1_w'], dtype=np.float32),
        conv1_b=np.asarray(inputs['conv1_b'], dtype=np.float32),
        conv2_w=np.asarray(inputs['conv2_w'], dtype=np.float32),
        conv2_b=np.asarray(inputs['conv2_b'], dtype=np.float32),
        fc_w=np.asarray(inputs['fc_w'], dtype=np.float32),
        fc_b=np.asarray(inputs['fc_b'], dtype=np.float32),
        C=8, trace=False, nc_cache=_NC_CACHE)
    return out
